# revision 7
# baseline (speedup 1.0000x reference)
"""Trainium2 Bass kernel for nn_CrossAttention (b=2, s1=2048, s2=3072, 16 heads, d=64).

Sharding: 8 cores = 2 batches x 4 head-groups (4 heads each). Each core:
  - computes q = LN(x @ WqT + bq)*scale, k = LN(y @ WkT + bk), v = y @ WvT + bv
    for its 4 heads from the full x[b] and the *valid-key-compacted* y[b],
  - computes scoresT = kT.T-free attention with the padding handled by a
    per-partition additive bias fused into the exp eviction (ACT),
  - accumulates ctxT via PE matmuls with v as the stationary operand; a ones
    column appended to v yields softmax denominators for free,
  - computes the partial output projection for its head group.
Host sums the 4 partials per batch and adds bo.
"""

import math
import os

import numpy as np

import concourse.bacc as bacc
import concourse.bass as bass
import concourse.tile as tile
from concourse import mybir
from concourse.bass_utils import run_bass_kernel_spmd
from concourse.masks import make_identity

F32 = mybir.dt.float32
F32R = mybir.dt.float32r
BF16 = mybir.dt.bfloat16

P = 128
D = 64
EPS = 1e-6
MASK_NEG = -1e9

# Matmul input dtype (f32r = 1 cycle/row for N>=256; f32 = 4 cycles/row).
MM_DT = F32R if os.environ.get("K_MM_DT", "f32r") == "f32r" else F32
# Probability / v dtype for the pv matmul.
PV_DT = BF16 if os.environ.get("K_PV_DT", "bf16") == "bf16" else F32

LAST_EXEC_NS = None


def _bcast_row(ap, nparts):
    """AP reading a (1, N) slice broadcast to (nparts, N) via a 0-stride
    partition dim (same trick as tile_groupnorm's bias load)."""
    return bass.AP(
        tensor=ap.tensor, offset=ap.offset, ap=[[0, nparts]] + list(ap.ap[1:])
    )


def _build_nc(S1, S2P, C, flags):
    G = 4 * D  # 256 channels per core (4 heads)
    NI = S1 // P
    NJ = S2P // P
    CT = C // P
    IBW = 1024 if NJ <= 16 else 512  # i-block width for the attention phase
    NIB = S1 // IBW
    NC2 = IBW // 512
    AF = mybir.ActivationFunctionType

    nc = bacc.Bacc("TRN2", target_bir_lowering=False, debug=False)

    xT_d = nc.dram_tensor("xT", [C, S1], MM_DT, kind="ExternalInput")
    yT_d = nc.dram_tensor("yT", [C, S2P], MM_DT, kind="ExternalInput")
    wqT_d = nc.dram_tensor("wqT", [C, G], MM_DT, kind="ExternalInput")
    wkT_d = nc.dram_tensor("wkT", [C, G], MM_DT, kind="ExternalInput")
    wvT_d = nc.dram_tensor("wvT", [C, G], MM_DT, kind="ExternalInput")
    woT_d = nc.dram_tensor("woT", [G, C], MM_DT, kind="ExternalInput")
    vec_d = nc.dram_tensor("vec", [8, G], F32, kind="ExternalInput")
    mask_d = nc.dram_tensor("maskb", [S2P], F32, kind="ExternalInput")
    out_d = nc.dram_tensor("out", [S1, C], F32, kind="ExternalOutput")

    VROW = {"bq": 0, "bk": 1, "bv": 2, "qw": 3, "qb": 4, "kw": 5, "kb": 6}

    with tile.TileContext(nc) as tc:
        with (
            tc.tile_pool(name="singles", bufs=1) as singles,
            tc.tile_pool(name="persist", bufs=1) as persist,
        ):
            ident = singles.tile([P, P], F32, tag="ident")
            make_identity(nc, ident)
            ones1 = singles.tile([1, D], MM_DT, tag="ones1")
            if MM_DT == F32:
                nc.vector.memset(ones1, 1.0)
            else:
                ones1f = singles.tile([1, D], F32, tag="ones1f")
                nc.vector.memset(ones1f, 1.0)
                nc.vector.tensor_copy(out=ones1, in_=ones1f)
            eps_sb = singles.tile([P, 1], F32, tag="eps")
            nc.vector.memset(eps_sb, EPS)
            mask_sb = singles.tile([P, NJ], F32, tag="mask")
            nc.gpsimd.dma_start(
                out=mask_sb, in_=mask_d[:].rearrange("(j p) -> p j", p=P)
            )
            vec_sb = {}
            for nm in [k for k, use in flags.items() if use]:
                t = singles.tile([P, G], F32, tag=f"vec_{nm}", name=f"vec_{nm}")
                nc.gpsimd.dma_start(
                    out=t, in_=_bcast_row(vec_d[VROW[nm] : VROW[nm] + 1, :], P)
                )
                vec_sb[nm] = t

            qT = [persist.tile([P, S1], MM_DT, tag=f"qT{i}", name=f"qT{i}") for i in range(2)]
            kT = [persist.tile([P, S2P], MM_DT, tag=f"kT{i}", name=f"kT{i}") for i in range(2)]
            v_sb = persist.tile([P, NJ, 4 * (D + 1)], PV_DT, tag="v")
            ctxT = [persist.tile([P, S1], MM_DT, tag=f"ctxT{i}", name=f"ctxT{i}") for i in range(2)]
            # ones column per head for the softmax denominator
            v4 = v_sb.rearrange("p j (h e) -> p j h e", e=D + 1)
            nc.vector.memset(v4[:, :, :, D : D + 1], 1.0)

            def ln_project(act_sb, w_sb, raw, mv, ntiles, scale_fold, bias_nm):
                """act_sb: (P, CT, S) transposed activations; produces raw
                (P, ntiles, G) = act.T @ W + bias and per-head mean/var."""
                for it in range(ntiles):
                    ps = psA.tile([P, G], F32, tag="psA")
                    for ct in range(CT):
                        nc.tensor.matmul(
                            ps,
                            lhsT=act_sb[:, ct, it * P : (it + 1) * P],
                            rhs=w_sb[:, ct, :],
                            start=(ct == 0),
                            stop=(ct == CT - 1),
                        )
                    dst = raw[:, it, :]
                    if bias_nm in vec_sb:
                        nc.vector.tensor_add(out=dst, in0=ps, in1=vec_sb[bias_nm])
                    else:
                        nc.vector.tensor_copy(out=dst, in_=ps)
                    for h4 in range(4):
                        st = work.tile([P, 6], F32, tag="bnst")
                        nc.vector.bn_stats(out=st, in_=dst[:, h4 * D : (h4 + 1) * D])
                        nc.vector.bn_aggr(out=mv[:, it, h4, :], in_=st)
                # batched rstd: rs = scale_fold / sqrt(var + eps)
                n4 = ntiles * 4
                mv_flat = mv.rearrange("p i h s -> p (i h s)")
                sd = work.tile([P, n4], F32, tag=f"sd{bias_nm}")
                nc.scalar.activation(
                    out=sd, in_=mv_flat[:, 1::2], func=AF.Sqrt, bias=eps_sb, scale=1.0
                )
                rs = work.tile([P, n4], F32, tag=f"rs{bias_nm}")
                nc.vector.reciprocal(out=rs, in_=sd)
                if scale_fold != 1.0:
                    nc.vector.tensor_scalar_mul(out=rs, in0=rs, scalar1=scale_fold)
                nm_ = work.tile([P, n4], F32, tag=f"nm{bias_nm}")
                nc.vector.tensor_mul(out=nm_, in0=mv_flat[:, 0::2], in1=rs)
                nc.vector.tensor_scalar_mul(out=nm_, in0=nm_, scalar1=-1.0)
                return rs, nm_

            def ln_apply_transpose(raw, rs, nm_, ntiles, w_nm, b_nm, dstT):
                for it in range(ntiles):
                    qa = work.tile([P, G], F32, tag="qa")
                    for h4 in range(4):
                        i4 = it * 4 + h4
                        nc.vector.tensor_scalar(
                            out=qa[:, h4 * D : (h4 + 1) * D],
                            in0=raw[:, it, h4 * D : (h4 + 1) * D],
                            scalar1=rs[:, i4 : i4 + 1],
                            scalar2=nm_[:, i4 : i4 + 1],
                            op0=mybir.AluOpType.mult,
                            op1=mybir.AluOpType.add,
                        )
                    if w_nm in vec_sb:
                        nc.vector.tensor_mul(out=qa, in0=qa, in1=vec_sb[w_nm])
                    if b_nm in vec_sb:
                        nc.vector.tensor_add(out=qa, in0=qa, in1=vec_sb[b_nm])
                    for half in range(2):
                        pt = psT.tile([P, P], F32, tag="ptr")
                        nc.tensor.transpose(pt, qa[:, half * P : (half + 1) * P], ident)
                        nc.vector.tensor_copy(
                            out=dstT[half][:, it * P : (it + 1) * P], in_=pt
                        )

            # ---------------- Phase 1: q projection + LN + transpose ---------
            with (
                tc.tile_pool(name="ph1", bufs=1) as ph1,
                tc.tile_pool(name="work", bufs=3) as work,
                tc.tile_pool(name="psA", bufs=3, space="PSUM") as psA,
                tc.tile_pool(name="psT", bufs=2, space="PSUM") as psT,
            ):
                xT_sb = ph1.tile([P, CT, S1], MM_DT, tag="xTs")
                xv = xT_d[:, :].rearrange("(ct p) i -> ct p i", p=P)
                for ct in range(CT):
                    nc.sync.dma_start(out=xT_sb[:, ct, :], in_=xv[ct])
                wq_sb = ph1.tile([P, CT, G], MM_DT, tag="wqs")
                wqv = wqT_d[:, :].rearrange("(ct p) g -> ct p g", p=P)
                for ct in range(CT):
                    nc.sync.dma_start(out=wq_sb[:, ct, :], in_=wqv[ct])

                qraw = ph1.tile([P, NI, G], F32, tag="qraw")
                mvq = ph1.tile([P, NI, 4, 2], F32, tag="mvq")
                rs_q, nm_q = ln_project(
                    xT_sb, wq_sb, qraw, mvq, NI, 1.0 / math.sqrt(D), "bq"
                )
                ln_apply_transpose(qraw, rs_q, nm_q, NI, "qw", "qb", qT)

            # ---------------- Phase 2: k/v projections ----------------------
            with (
                tc.tile_pool(name="ph2", bufs=1) as ph2,
                tc.tile_pool(name="work", bufs=3) as work,
                tc.tile_pool(name="psA", bufs=3, space="PSUM") as psA,
                tc.tile_pool(name="psT", bufs=2, space="PSUM") as psT,
            ):
                yT_sb = ph2.tile([P, CT, S2P], MM_DT, tag="yTs")
                yv = yT_d[:, :].rearrange("(ct p) j -> ct p j", p=P)
                for ct in range(CT):
                    nc.sync.dma_start(out=yT_sb[:, ct, :], in_=yv[ct])
                wk_sb = ph2.tile([P, CT, G], MM_DT, tag="wks")
                wv_sb = ph2.tile([P, CT, G], MM_DT, tag="wvs")
                wkv = wkT_d[:, :].rearrange("(ct p) g -> ct p g", p=P)
                wvv = wvT_d[:, :].rearrange("(ct p) g -> ct p g", p=P)
                for ct in range(CT):
                    nc.sync.dma_start(out=wk_sb[:, ct, :], in_=wkv[ct])
                    nc.sync.dma_start(out=wv_sb[:, ct, :], in_=wvv[ct])

                kraw = ph2.tile([P, NJ, G], F32, tag="kraw")
                mvk = ph2.tile([P, NJ, 4, 2], F32, tag="mvk")
                rs_k, nm_k = ln_project(yT_sb, wk_sb, kraw, mvk, NJ, 1.0, "bk")
                ln_apply_transpose(kraw, rs_k, nm_k, NJ, "kw", "kb", kT)

                # v projection (no LN, no transpose; strided 65-col layout)
                for jt in range(NJ):
                    ps = psA.tile([P, G], F32, tag="psA")
                    for ct in range(CT):
                        nc.tensor.matmul(
                            ps,
                            lhsT=yT_sb[:, ct, jt * P : (jt + 1) * P],
                            rhs=wv_sb[:, ct, :],
                            start=(ct == 0),
                            stop=(ct == CT - 1),
                        )
                    ps3 = ps.rearrange("p (h e) -> p h e", e=D)
                    vdst = v4[:, jt, :, 0:D]
                    if "bv" in vec_sb:
                        bv3 = vec_sb["bv"].rearrange("p (h e) -> p h e", e=D)
                        nc.vector.tensor_add(out=vdst, in0=ps3, in1=bv3)
                    else:
                        nc.vector.tensor_copy(out=vdst, in_=ps3)

            # ---------------- Phase 3: attention ----------------------------
            with (
                tc.tile_pool(name="pp", bufs=2) as ppool,
                tc.tile_pool(name="attw", bufs=3) as attw,
                tc.tile_pool(name="psS", bufs=2, space="PSUM") as psS,
                tc.tile_pool(name="psC", bufs=2, space="PSUM") as psC,
                tc.tile_pool(name="psB", bufs=2, space="PSUM") as psB,
            ):
                for ib in range(NIB):
                    for hp in range(2):
                        pts = [
                            ppool.tile([P, NJ, IBW], PV_DT, tag=f"p{h2}", name=f"p{h2}")
                            for h2 in range(2)
                        ]
                        for jt in range(NJ):
                            for h2 in range(2):
                                ps = psS.tile([P, IBW], F32, tag="ps_s")
                                for cc in range(NC2):
                                    c0 = ib * IBW + cc * 512
                                    nc.tensor.matmul(
                                        ps[:, cc * 512 : (cc + 1) * 512],
                                        lhsT=kT[hp][
                                            h2 * D : (h2 + 1) * D,
                                            jt * P : (jt + 1) * P,
                                        ],
                                        rhs=qT[hp][
                                            h2 * D : (h2 + 1) * D, c0 : c0 + 512
                                        ],
                                        start=True,
                                        stop=True,
                                    )
                                nc.scalar.activation(
                                    out=pts[h2][:, jt, :],
                                    in_=ps,
                                    func=AF.Exp,
                                    bias=mask_sb[:, jt : jt + 1],
                                    scale=1.0,
                                )
                        for h2 in range(2):
                            hg = hp * 2 + h2
                            for cc in range(NC2):
                                pc = psC.tile([D + 1, 512], F32, tag="ps_c")
                                for jt in range(NJ):
                                    nc.tensor.matmul(
                                        pc,
                                        lhsT=v_sb[
                                            :, jt, hg * (D + 1) : (hg + 1) * (D + 1)
                                        ],
                                        rhs=pts[h2][:, jt, cc * 512 : (cc + 1) * 512],
                                        start=(jt == 0),
                                        stop=(jt == NJ - 1),
                                    )
                                den = attw.tile([1, 512], MM_DT, tag="den")
                                nc.vector.tensor_copy(out=den, in_=pc[D : D + 1, :])
                                pb = psB.tile([D, 512], F32, tag="ps_b")
                                nc.tensor.matmul(
                                    pb,
                                    lhsT=ones1,
                                    rhs=den,
                                    start=True,
                                    stop=True,
                                )
                                rec = attw.tile([D, 512], F32, tag="rec")
                                nc.vector.reciprocal(out=rec, in_=pb)
                                c0 = ib * IBW + cc * 512
                                nc.vector.tensor_mul(
                                    out=ctxT[hp][h2 * D : (h2 + 1) * D, c0 : c0 + 512],
                                    in0=pc[0:D, :],
                                    in1=rec,
                                )

            # ---------------- Phase 4: output projection --------------------
            with (
                tc.tile_pool(name="ph4", bufs=1) as ph4,
                tc.tile_pool(name="ow", bufs=3) as ow,
                tc.tile_pool(name="psO", bufs=2, space="PSUM") as psO,
            ):
                wo_sb = ph4.tile([P, 2, C], MM_DT, tag="wo")
                wov = woT_d[:, :].rearrange("(k p) c -> k p c", p=P)
                for kt in range(2):
                    nc.sync.dma_start(out=wo_sb[:, kt, :], in_=wov[kt])
                for it in range(NI):
                    po = psO.tile([P, C], F32, tag="ps_o")
                    for oc in range(C // 512):
                        for kt in range(2):
                            nc.tensor.matmul(
                                po[:, oc * 512 : (oc + 1) * 512],
                                lhsT=ctxT[kt][:, it * P : (it + 1) * P],
                                rhs=wo_sb[:, kt, oc * 512 : (oc + 1) * 512].bitcast(
                                    MM_DT
                                ),
                                start=(kt == 0),
                                stop=(kt == 1),
                            )
                    ot = ow.tile([P, C], F32, tag="ot")
                    nc.vector.tensor_copy(out=ot, in_=po)
                    nc.sync.dma_start(out=out_d[it * P : (it + 1) * P, :], in_=ot)

    nc.finalize()
    return nc


def kernel(x, y, padding_mask, Wq, bq, Wkv, bkv, qn_w, qn_b, kn_w, kn_b, Wo, bo):
    global LAST_EXEC_NS
    x = np.asarray(x, dtype=np.float32)
    y = np.asarray(y, dtype=np.float32)
    padding_mask = np.asarray(padding_mask)
    Wq = np.asarray(Wq, dtype=np.float32)
    bq = np.asarray(bq, dtype=np.float32)
    Wkv = np.asarray(Wkv, dtype=np.float32)
    bkv = np.asarray(bkv, dtype=np.float32)
    qn_w = np.asarray(qn_w, dtype=np.float32)
    qn_b = np.asarray(qn_b, dtype=np.float32)
    kn_w = np.asarray(kn_w, dtype=np.float32)
    kn_b = np.asarray(kn_b, dtype=np.float32)
    Wo = np.asarray(Wo, dtype=np.float32)
    bo = np.asarray(bo, dtype=np.float32)

    b, S1, C = x.shape
    assert b == 2 and C % 16 == 0
    d = C // 16
    scale = d ** -0.5
    G = 4 * d  # 4 heads per core

    idxs = [np.flatnonzero(padding_mask[bi]) for bi in range(b)]
    s2v = [len(ix) for ix in idxs]
    S2P = max(P, ((max(s2v) + P - 1) // P) * P)

    flags = {
        "bq": bool(np.any(bq)),
        "bk": bool(np.any(bkv[:C])),
        "bv": bool(np.any(bkv[C:])),
        "qw": not bool(np.all(qn_w == 1.0)),
        "qb": bool(np.any(qn_b)),
        "kw": not bool(np.all(kn_w == 1.0)),
        "kb": bool(np.any(kn_b)),
    }

    nc = _build_nc(S1, S2P, C, flags)

    in_maps = []
    yTs = []
    for bi in range(b):
        yv = np.zeros((S2P, C), np.float32)
        yv[: s2v[bi]] = y[bi][idxs[bi]]
        yTs.append(np.ascontiguousarray(yv.T))
    xTs = [np.ascontiguousarray(x[bi].T) for bi in range(b)]
    for core in range(8):
        bc, g = divmod(core, 4)
        rows = slice(g * G, (g + 1) * G)
        vecs = np.zeros((8, G), np.float32)
        vecs[0] = bq[rows]
        vecs[1] = bkv[rows]
        vecs[2] = bkv[C + g * G : C + (g + 1) * G]
        vecs[3] = np.tile(qn_w, 4)
        vecs[4] = np.tile(qn_b * scale, 4)
        vecs[5] = np.tile(kn_w, 4)
        vecs[6] = np.tile(kn_b, 4)
        mb = np.zeros((S2P,), np.float32)
        mb[s2v[bc] :] = MASK_NEG
        in_maps.append(
            {
                "xT": xTs[bc],
                "yT": yTs[bc],
                "wqT": np.ascontiguousarray(Wq[rows, :].T),
                "wkT": np.ascontiguousarray(Wkv[rows, :].T),
                "wvT": np.ascontiguousarray(Wkv[C + g * G : C + (g + 1) * G, :].T),
                "woT": np.ascontiguousarray(Wo[:, rows].T),
                "vec": vecs,
                "maskb": mb,
            }
        )

    res = run_bass_kernel_spmd(nc, in_maps, core_ids=list(range(8)))
    LAST_EXEC_NS = res.exec_time_ns

    out = np.zeros((b, S1, C), np.float32)
    for core in range(8):
        out[core // 4] += res.results[core]["out"]
    out += bo
    return out


# revision 12
# speedup vs baseline: 1.1765x; 1.1765x over previous
"""Trainium2 Bass kernel for nn_CrossAttention (b=2, s1=2048, s2=3072, 16 heads, d=64).

Sharding: 8 cores = 2 batches x 4 head-groups (4 heads each). Each core:
  - computes q = LN(x @ WqT + bq)*scale, k = LN(y @ WkT + bk), v = y @ WvT + bv
    for its 4 heads from the full x[b] and the *valid-key-compacted* y[b],
  - computes scoresT = kT.T-free attention with the padding handled by a
    per-partition additive bias fused into the exp eviction (ACT),
  - accumulates ctxT via PE matmuls with v as the stationary operand; a ones
    column appended to v yields softmax denominators for free,
  - computes the partial output projection for its head group.
Host sums the 4 partials per batch and adds bo.
"""

import math
import os

import ml_dtypes  # noqa: F401  (np bfloat16 support)
import numpy as np


import concourse.bacc as bacc
import concourse.bass as bass
import concourse.tile as tile
from concourse import mybir
from concourse.bass_utils import run_bass_kernel_spmd
from concourse.masks import make_identity

F32 = mybir.dt.float32
F32R = mybir.dt.float32r
BF16 = mybir.dt.bfloat16

P = 128
D = 64
EPS = 1e-6
MASK_NEG = -1e9

# Matmul input dtype: bf16 = 1 cycle/row + FWL; f32r lowers to fp32-HIGH at
# ~2 cycles/row; f32 = 4 cycles/row.
MM_DT = {"bf16": BF16, "f32r": F32R, "f32": F32}[os.environ.get("K_MM_DT", "bf16")]
# Probability / v dtype for the pv matmul.
PV_DT = BF16 if os.environ.get("K_PV_DT", "bf16") == "bf16" else F32

LAST_EXEC_NS = None


def _bcast_row(ap, nparts):
    """AP reading a (1, N) slice broadcast to (nparts, N) via a 0-stride
    partition dim (same trick as tile_groupnorm's bias load)."""
    return bass.AP(
        tensor=ap.tensor, offset=ap.offset, ap=[[0, nparts]] + list(ap.ap[1:])
    )


def _build_nc(S1, S2P, C, flags):
    G = 4 * D  # 256 channels per core (4 heads)
    NI = S1 // P
    NJ = S2P // P
    CT = C // P
    IBW = 1024 if NJ <= 16 else 512  # i-block width for the attention phase
    NIB = S1 // IBW
    NC2 = IBW // 512
    AF = mybir.ActivationFunctionType

    TR_DT = BF16 if MM_DT == BF16 else F32  # LN-output / transpose dtype

    nc = bacc.Bacc("TRN2", target_bir_lowering=False, debug=False)

    xT_d = nc.dram_tensor("xT", [C, S1], MM_DT, kind="ExternalInput")
    yT_d = nc.dram_tensor("yT", [C, S2P], MM_DT, kind="ExternalInput")
    wqT_d = nc.dram_tensor("wqT", [C, G], MM_DT, kind="ExternalInput")
    wkT_d = nc.dram_tensor("wkT", [C, G], MM_DT, kind="ExternalInput")
    wvT_d = nc.dram_tensor("wvT", [C, G], MM_DT, kind="ExternalInput")
    woT_d = nc.dram_tensor("woT", [G, C], MM_DT, kind="ExternalInput")
    vec_d = nc.dram_tensor("vec", [8, G], F32, kind="ExternalInput")
    mask_d = nc.dram_tensor("maskb", [S2P], F32, kind="ExternalInput")
    out_d = nc.dram_tensor("out", [S1, C], F32, kind="ExternalOutput")

    VROW = {"bq": 0, "bk": 1, "bv": 2, "qw": 3, "qb": 4, "kw": 5, "kb": 6}

    with tile.TileContext(nc) as tc:
        with (
            tc.tile_pool(name="singles", bufs=1) as singles,
            tc.tile_pool(name="persist", bufs=1) as persist,
        ):
            ident = singles.tile([P, P], TR_DT, tag="ident")
            make_identity(nc, ident)
            eps_sb = singles.tile([P, 1], F32, tag="eps")
            nc.vector.memset(eps_sb, EPS)
            mask_sb = singles.tile([P, NJ], F32, tag="mask")
            nc.gpsimd.dma_start(
                out=mask_sb, in_=mask_d[:].rearrange("(j p) -> p j", p=P)
            )
            vec_sb = {}
            for nm in [k for k, use in flags.items() if use]:
                t = singles.tile([P, G], F32, tag=f"vec_{nm}", name=f"vec_{nm}")
                nc.gpsimd.dma_start(
                    out=t, in_=_bcast_row(vec_d[VROW[nm] : VROW[nm] + 1, :], P)
                )
                vec_sb[nm] = t

            qT = [persist.tile([P, S1], MM_DT, tag=f"qT{i}", name=f"qT{i}") for i in range(2)]
            kT = [persist.tile([P, S2P], MM_DT, tag=f"kT{i}", name=f"kT{i}") for i in range(2)]
            v_sb = persist.tile([P, NJ, 4 * (D + 1)], PV_DT, tag="v")
            ctxT = [persist.tile([P, S1], MM_DT, tag=f"ctxT{i}", name=f"ctxT{i}") for i in range(2)]
            # ones column per head for the softmax denominator
            v4 = v_sb.rearrange("p j (h e) -> p j h e", e=D + 1)
            nc.vector.memset(v4[:, :, :, D : D + 1], 1.0)

            def ln_project(act_sb, w_sb, raw, mv, ntiles, scale_fold, bias_nm):
                """act_sb: (P, CT, S) transposed activations; produces raw
                (P, ntiles, G) = act.T @ W + bias and per-head mean/var."""
                for it in range(ntiles):
                    ps = psA.tile([P, G], F32, tag="psA")
                    for ct in range(CT):
                        nc.tensor.matmul(
                            ps,
                            lhsT=act_sb[:, ct, it * P : (it + 1) * P],
                            rhs=w_sb[:, ct, :],
                            start=(ct == 0),
                            stop=(ct == CT - 1),
                        )
                    dst = raw[:, it, :]
                    if bias_nm in vec_sb:
                        nc.vector.tensor_add(out=dst, in0=ps, in1=vec_sb[bias_nm])
                    else:
                        nc.vector.tensor_copy(out=dst, in_=ps)
                    for h4 in range(4):
                        st = work.tile([P, 6], F32, tag="bnst")
                        nc.vector.bn_stats(out=st, in_=dst[:, h4 * D : (h4 + 1) * D])
                        nc.vector.bn_aggr(out=mv[:, it, h4, :], in_=st)
                # batched rstd: rs = scale_fold / sqrt(var + eps)
                n4 = ntiles * 4
                mv_flat = mv.rearrange("p i h s -> p (i h s)")
                sd = work.tile([P, n4], F32, tag=f"sd{bias_nm}")
                nc.scalar.activation(
                    out=sd, in_=mv_flat[:, 1::2], func=AF.Sqrt, bias=eps_sb, scale=1.0
                )
                rs = work.tile([P, n4], F32, tag=f"rs{bias_nm}")
                nc.vector.reciprocal(out=rs, in_=sd)
                if scale_fold != 1.0:
                    nc.vector.tensor_scalar_mul(out=rs, in0=rs, scalar1=scale_fold)
                nm_ = work.tile([P, n4], F32, tag=f"nm{bias_nm}")
                nc.vector.tensor_mul(out=nm_, in0=mv_flat[:, 0::2], in1=rs)
                nc.vector.tensor_scalar_mul(out=nm_, in0=nm_, scalar1=-1.0)
                return rs, nm_

            def ln_apply_transpose(raw, rs, nm_, ntiles, w_nm, b_nm, dstT):
                for it in range(ntiles):
                    qa = work.tile([P, G], TR_DT, tag="qa")
                    for h4 in range(4):
                        i4 = it * 4 + h4
                        nc.vector.tensor_scalar(
                            out=qa[:, h4 * D : (h4 + 1) * D],
                            in0=raw[:, it, h4 * D : (h4 + 1) * D],
                            scalar1=rs[:, i4 : i4 + 1],
                            scalar2=nm_[:, i4 : i4 + 1],
                            op0=mybir.AluOpType.mult,
                            op1=mybir.AluOpType.add,
                        )
                    if w_nm in vec_sb:
                        nc.vector.tensor_mul(out=qa, in0=qa, in1=vec_sb[w_nm])
                    if b_nm in vec_sb:
                        nc.vector.tensor_add(out=qa, in0=qa, in1=vec_sb[b_nm])
                    for half in range(2):
                        pt = psT.tile([P, P], TR_DT, tag="ptr")
                        nc.tensor.transpose(pt, qa[:, half * P : (half + 1) * P], ident)
                        nc.vector.tensor_copy(
                            out=dstT[half][:, it * P : (it + 1) * P], in_=pt
                        )

            # ---------------- Phase 1: q projection + LN + transpose ---------
            with (
                tc.tile_pool(name="ph1", bufs=1) as ph1,
                tc.tile_pool(name="work", bufs=3) as work,
                tc.tile_pool(name="psA", bufs=3, space="PSUM") as psA,
                tc.tile_pool(name="psT", bufs=2, space="PSUM") as psT,
            ):
                xT_sb = ph1.tile([P, CT, S1], MM_DT, tag="xTs")
                xv = xT_d[:, :].rearrange("(ct p) i -> ct p i", p=P)
                for ct in range(CT):
                    nc.sync.dma_start(out=xT_sb[:, ct, :], in_=xv[ct])
                wq_sb = ph1.tile([P, CT, G], MM_DT, tag="wqs")
                wqv = wqT_d[:, :].rearrange("(ct p) g -> ct p g", p=P)
                for ct in range(CT):
                    nc.sync.dma_start(out=wq_sb[:, ct, :], in_=wqv[ct])

                qraw = ph1.tile([P, NI, G], F32, tag="qraw")
                mvq = ph1.tile([P, NI, 4, 2], F32, tag="mvq")
                rs_q, nm_q = ln_project(
                    xT_sb, wq_sb, qraw, mvq, NI, 1.0 / math.sqrt(D), "bq"
                )
                ln_apply_transpose(qraw, rs_q, nm_q, NI, "qw", "qb", qT)

            # ---------------- Phase 2: k/v projections ----------------------
            with (
                tc.tile_pool(name="ph2", bufs=1) as ph2,
                tc.tile_pool(name="work", bufs=3) as work,
                tc.tile_pool(name="psA", bufs=3, space="PSUM") as psA,
                tc.tile_pool(name="psT", bufs=2, space="PSUM") as psT,
            ):
                yT_sb = ph2.tile([P, CT, S2P], MM_DT, tag="yTs")
                yv = yT_d[:, :].rearrange("(ct p) j -> ct p j", p=P)
                for ct in range(CT):
                    nc.sync.dma_start(out=yT_sb[:, ct, :], in_=yv[ct])
                wk_sb = ph2.tile([P, CT, G], MM_DT, tag="wks")
                wv_sb = ph2.tile([P, CT, G], MM_DT, tag="wvs")
                wkv = wkT_d[:, :].rearrange("(ct p) g -> ct p g", p=P)
                wvv = wvT_d[:, :].rearrange("(ct p) g -> ct p g", p=P)
                for ct in range(CT):
                    nc.sync.dma_start(out=wk_sb[:, ct, :], in_=wkv[ct])
                    nc.sync.dma_start(out=wv_sb[:, ct, :], in_=wvv[ct])

                kraw = ph2.tile([P, NJ, G], F32, tag="kraw")
                mvk = ph2.tile([P, NJ, 4, 2], F32, tag="mvk")
                rs_k, nm_k = ln_project(yT_sb, wk_sb, kraw, mvk, NJ, 1.0, "bk")
                ln_apply_transpose(kraw, rs_k, nm_k, NJ, "kw", "kb", kT)

                # v projection (no LN, no transpose; strided 65-col layout)
                for jt in range(NJ):
                    ps = psA.tile([P, G], F32, tag="psA")
                    for ct in range(CT):
                        nc.tensor.matmul(
                            ps,
                            lhsT=yT_sb[:, ct, jt * P : (jt + 1) * P],
                            rhs=wv_sb[:, ct, :],
                            start=(ct == 0),
                            stop=(ct == CT - 1),
                        )
                    ps3 = ps.rearrange("p (h e) -> p h e", e=D)
                    vdst = v4[:, jt, :, 0:D]
                    if "bv" in vec_sb:
                        bv3 = vec_sb["bv"].rearrange("p (h e) -> p h e", e=D)
                        nc.vector.tensor_add(out=vdst, in0=ps3, in1=bv3)
                    else:
                        nc.vector.tensor_copy(out=vdst, in_=ps3)

            # ---------------- Phase 3: attention ----------------------------
            with (
                tc.tile_pool(name="pp", bufs=2) as ppool,
                tc.tile_pool(name="attw", bufs=3) as attw,
                tc.tile_pool(name="attden", bufs=1) as attden,
                tc.tile_pool(name="dram", bufs=1, space="DRAM") as dramp,
                tc.tile_pool(name="psS", bufs=2, space="PSUM") as psS,
                tc.tile_pool(name="psC", bufs=1, space="PSUM") as psC,
            ):
                NU = NIB * 2 * 2 * NC2  # normalization units (h2 x cc per ib,hp)
                rec_dram = dramp.tile([NU, 512], F32, tag="rec_dram")
                for ib in range(NIB):
                    for hp in range(2):
                        pts = [
                            ppool.tile([P, NJ, IBW], PV_DT, tag=f"p{h2}", name=f"p{h2}")
                            for h2 in range(2)
                        ]
                        for jt in range(NJ):
                            for h2 in range(2):
                                ps = psS.tile([P, IBW], F32, tag="ps_s")
                                for cc in range(NC2):
                                    c0 = ib * IBW + cc * 512
                                    nc.tensor.matmul(
                                        ps[:, cc * 512 : (cc + 1) * 512],
                                        lhsT=kT[hp][
                                            h2 * D : (h2 + 1) * D,
                                            jt * P : (jt + 1) * P,
                                        ],
                                        rhs=qT[hp][
                                            h2 * D : (h2 + 1) * D, c0 : c0 + 512
                                        ],
                                        start=True,
                                        stop=True,
                                    )
                                nc.scalar.activation(
                                    out=pts[h2][:, jt, :],
                                    in_=ps,
                                    func=AF.Exp,
                                    bias=mask_sb[:, jt : jt + 1],
                                    scale=1.0,
                                )
                        pcs = {}
                        u0 = (ib * 2 + hp) * 2 * NC2
                        den_blk = attw.tile([P, 512], F32, tag="den_blk")
                        nc.vector.memset(den_blk, 1.0)
                        for h2 in range(2):
                            hg = hp * 2 + h2
                            for cc in range(NC2):
                                pc = psC.tile(
                                    [D + 1, 512], F32, tag=f"ps_c{h2}{cc}", name="pc"
                                )
                                pcs[(h2, cc)] = pc
                                for jt in range(NJ):
                                    nc.tensor.matmul(
                                        pc,
                                        lhsT=v_sb[
                                            :, jt, hg * (D + 1) : (hg + 1) * (D + 1)
                                        ],
                                        rhs=pts[h2][:, jt, cc * 512 : (cc + 1) * 512],
                                        start=(jt == 0),
                                        stop=(jt == NJ - 1),
                                    )
                                uu = h2 * NC2 + cc
                                nc.vector.tensor_copy(
                                    out=den_blk[32 * uu : 32 * uu + 1, :],
                                    in_=pc[D : D + 1, :],
                                )
                        # one batched reciprocal per (ib, hp) block (rows at
                        # 32-aligned partitions), bounced via DRAM so each row
                        # can partition-broadcast on the way back in
                        rec_blk = attw.tile([P, 512], F32, tag="rec_blk")
                        nc.vector.reciprocal(out=rec_blk, in_=den_blk)
                        rec_rows = rec_blk.rearrange("(a b) f -> a b f", b=32)[:, 0, :]
                        nc.gpsimd.dma_start(
                            out=rec_dram[u0 : u0 + 2 * NC2, :], in_=rec_rows
                        )
                        for h2 in range(2):
                            for cc in range(NC2):
                                u = u0 + h2 * NC2 + cc
                                rec = attw.tile([D, 512], F32, tag="rec")
                                nc.gpsimd.dma_start(
                                    out=rec, in_=_bcast_row(rec_dram[u : u + 1, :], D)
                                )
                                c0 = ib * IBW + cc * 512
                                nc.vector.tensor_mul(
                                    out=ctxT[hp][h2 * D : (h2 + 1) * D, c0 : c0 + 512],
                                    in0=pcs[(h2, cc)][0:D, :],
                                    in1=rec,
                                )

            # ---------------- Phase 4: output projection --------------------
            with (
                tc.tile_pool(name="ph4", bufs=1) as ph4,
                tc.tile_pool(name="ow", bufs=3) as ow,
                tc.tile_pool(name="psO", bufs=2, space="PSUM") as psO,
            ):
                wo_sb = ph4.tile([P, 2, C], MM_DT, tag="wo")
                wov = woT_d[:, :].rearrange("(k p) c -> k p c", p=P)
                for kt in range(2):
                    nc.sync.dma_start(out=wo_sb[:, kt, :], in_=wov[kt])
                for it in range(NI):
                    po = psO.tile([P, C], F32, tag="ps_o")
                    for oc in range(C // 512):
                        for kt in range(2):
                            nc.tensor.matmul(
                                po[:, oc * 512 : (oc + 1) * 512],
                                lhsT=ctxT[kt][:, it * P : (it + 1) * P],
                                rhs=wo_sb[:, kt, oc * 512 : (oc + 1) * 512].bitcast(
                                    MM_DT
                                ),
                                start=(kt == 0),
                                stop=(kt == 1),
                            )
                    ot = ow.tile([P, C], F32, tag="ot")
                    nc.vector.tensor_copy(out=ot, in_=po)
                    nc.sync.dma_start(out=out_d[it * P : (it + 1) * P, :], in_=ot)

    nc.finalize()
    return nc


def kernel(x, y, padding_mask, Wq, bq, Wkv, bkv, qn_w, qn_b, kn_w, kn_b, Wo, bo):
    global LAST_EXEC_NS
    x = np.asarray(x, dtype=np.float32)
    y = np.asarray(y, dtype=np.float32)
    padding_mask = np.asarray(padding_mask)
    Wq = np.asarray(Wq, dtype=np.float32)
    bq = np.asarray(bq, dtype=np.float32)
    Wkv = np.asarray(Wkv, dtype=np.float32)
    bkv = np.asarray(bkv, dtype=np.float32)
    qn_w = np.asarray(qn_w, dtype=np.float32)
    qn_b = np.asarray(qn_b, dtype=np.float32)
    kn_w = np.asarray(kn_w, dtype=np.float32)
    kn_b = np.asarray(kn_b, dtype=np.float32)
    Wo = np.asarray(Wo, dtype=np.float32)
    bo = np.asarray(bo, dtype=np.float32)

    b, S1, C = x.shape
    assert b == 2 and C % 16 == 0
    d = C // 16
    scale = d ** -0.5
    G = 4 * d  # 4 heads per core

    idxs = [np.flatnonzero(padding_mask[bi]) for bi in range(b)]
    s2v = [len(ix) for ix in idxs]
    S2P = max(P, ((max(s2v) + P - 1) // P) * P)

    flags = {
        "bq": bool(np.any(bq)),
        "bk": bool(np.any(bkv[:C])),
        "bv": bool(np.any(bkv[C:])),
        "qw": not bool(np.all(qn_w == 1.0)),
        "qb": bool(np.any(qn_b)),
        "kw": not bool(np.all(kn_w == 1.0)),
        "kb": bool(np.any(kn_b)),
    }

    nc = _build_nc(S1, S2P, C, flags)

    mm_np = {BF16: ml_dtypes.bfloat16, F32R: np.float32, F32: np.float32}[MM_DT]
    in_maps = []
    yTs = []
    for bi in range(b):
        yv = np.zeros((S2P, C), np.float32)
        yv[: s2v[bi]] = y[bi][idxs[bi]]
        yTs.append(np.ascontiguousarray(yv.T).astype(mm_np))
    xTs = [np.ascontiguousarray(x[bi].T).astype(mm_np) for bi in range(b)]
    for core in range(8):
        bc, g = divmod(core, 4)
        rows = slice(g * G, (g + 1) * G)
        vecs = np.zeros((8, G), np.float32)
        vecs[0] = bq[rows]
        vecs[1] = bkv[rows]
        vecs[2] = bkv[C + g * G : C + (g + 1) * G]
        vecs[3] = np.tile(qn_w, 4)
        vecs[4] = np.tile(qn_b * scale, 4)
        vecs[5] = np.tile(kn_w, 4)
        vecs[6] = np.tile(kn_b, 4)
        mb = np.zeros((S2P,), np.float32)
        mb[s2v[bc] :] = MASK_NEG
        in_maps.append(
            {
                "xT": xTs[bc],
                "yT": yTs[bc],
                "wqT": np.ascontiguousarray(Wq[rows, :].T).astype(mm_np),
                "wkT": np.ascontiguousarray(Wkv[rows, :].T).astype(mm_np),
                "wvT": np.ascontiguousarray(
                    Wkv[C + g * G : C + (g + 1) * G, :].T
                ).astype(mm_np),
                "woT": np.ascontiguousarray(Wo[:, rows].T).astype(mm_np),
                "vec": vecs,
                "maskb": mb,
            }
        )

    res = run_bass_kernel_spmd(nc, in_maps, core_ids=list(range(8)))
    LAST_EXEC_NS = res.exec_time_ns

    out = np.zeros((b, S1, C), np.float32)
    for core in range(8):
        out[core // 4] += res.results[core]["out"]
    out += bo
    return out


# revision 13
# speedup vs baseline: 1.4019x; 1.1915x over previous
"""Trainium2 Bass kernel for nn_CrossAttention (b=2, s1=2048, s2=3072, 16 heads, d=64).

Sharding: 8 cores = 2 batches x 4 head-groups (4 heads each). Each core:
  - computes q = LN(x @ WqT + bq)*scale, k = LN(y @ WkT + bk), v = y @ WvT + bv
    for its 4 heads from the full x[b] and the *valid-key-compacted* y[b],
  - computes scoresT = kT.T-free attention with the padding handled by a
    per-partition additive bias fused into the exp eviction (ACT),
  - accumulates ctxT via PE matmuls with v as the stationary operand; a ones
    column appended to v yields softmax denominators for free,
  - computes the partial output projection for its head group.
Host sums the 4 partials per batch and adds bo.
"""

import math
import os

import ml_dtypes  # noqa: F401  (np bfloat16 support)
import numpy as np


import concourse.bacc as bacc
import concourse.bass as bass
import concourse.tile as tile
from concourse import mybir
from concourse.bass_utils import run_bass_kernel_spmd
from concourse.masks import make_identity

F32 = mybir.dt.float32
F32R = mybir.dt.float32r
BF16 = mybir.dt.bfloat16

P = 128
D = 64
EPS = 1e-6
MASK_NEG = -1e9

# Matmul input dtype: bf16 = 1 cycle/row + FWL; f32r lowers to fp32-HIGH at
# ~2 cycles/row; f32 = 4 cycles/row.
MM_DT = {"bf16": BF16, "f32r": F32R, "f32": F32}[os.environ.get("K_MM_DT", "bf16")]
# Probability / v dtype for the pv matmul.
PV_DT = BF16 if os.environ.get("K_PV_DT", "bf16") == "bf16" else F32

LAST_EXEC_NS = None


def _bcast_row(ap, nparts):
    """AP reading a (1, N) slice broadcast to (nparts, N) via a 0-stride
    partition dim (same trick as tile_groupnorm's bias load)."""
    return bass.AP(
        tensor=ap.tensor, offset=ap.offset, ap=[[0, nparts]] + list(ap.ap[1:])
    )


def _build_nc(S1, S2P, C, flags):
    G = 4 * D  # 256 channels per core (4 heads)
    NI = S1 // P
    NJ = S2P // P
    CT = C // P
    IBW = 1024 if NJ <= 16 else 512  # i-block width for the attention phase
    NIB = S1 // IBW
    NC2 = IBW // 512
    AF = mybir.ActivationFunctionType

    TR_DT = BF16 if MM_DT == BF16 else F32  # LN-output / transpose dtype

    nc = bacc.Bacc("TRN2", target_bir_lowering=False, debug=False)

    xT_d = nc.dram_tensor("xT", [C, S1], MM_DT, kind="ExternalInput")
    yT_d = nc.dram_tensor("yT", [C, S2P], MM_DT, kind="ExternalInput")
    wqT_d = nc.dram_tensor("wqT", [C, G], MM_DT, kind="ExternalInput")
    wkT_d = nc.dram_tensor("wkT", [C, G], MM_DT, kind="ExternalInput")
    wvT_d = nc.dram_tensor("wvT", [C, G], MM_DT, kind="ExternalInput")
    woT_d = nc.dram_tensor("woT", [G, C], MM_DT, kind="ExternalInput")
    vec_d = nc.dram_tensor("vec", [8, G], F32, kind="ExternalInput")
    mask_d = nc.dram_tensor("maskb", [S2P], F32, kind="ExternalInput")
    out_d = nc.dram_tensor("out", [S1, C], F32, kind="ExternalOutput")

    VROW = {"bq": 0, "bk": 1, "bv": 2, "qw": 3, "qb": 4, "kw": 5, "kb": 6}

    with tile.TileContext(nc) as tc:
        with (
            tc.tile_pool(name="singles", bufs=1) as singles,
            tc.tile_pool(name="persist", bufs=1) as persist,
        ):
            ident = singles.tile([P, P], TR_DT, tag="ident")
            make_identity(nc, ident)
            eps_sb = singles.tile([P, 1], F32, tag="eps")
            nc.vector.memset(eps_sb, EPS)
            mask_sb = singles.tile([P, NJ], F32, tag="mask")
            nc.gpsimd.dma_start(
                out=mask_sb, in_=mask_d[:].rearrange("(j p) -> p j", p=P)
            )
            vec_sb = {}
            for nm in [k for k, use in flags.items() if use]:
                t = singles.tile([P, G], F32, tag=f"vec_{nm}", name=f"vec_{nm}")
                nc.gpsimd.dma_start(
                    out=t, in_=_bcast_row(vec_d[VROW[nm] : VROW[nm] + 1, :], P)
                )
                vec_sb[nm] = t

            qT = [persist.tile([P, S1], MM_DT, tag=f"qT{i}", name=f"qT{i}") for i in range(2)]
            kT = [persist.tile([P, S2P], MM_DT, tag=f"kT{i}", name=f"kT{i}") for i in range(2)]
            v_sb = persist.tile([P, NJ, 4 * (D + 1)], PV_DT, tag="v")
            ctxT = [persist.tile([P, S1], MM_DT, tag=f"ctxT{i}", name=f"ctxT{i}") for i in range(2)]
            # ones column per head for the softmax denominator
            v4 = v_sb.rearrange("p j (h e) -> p j h e", e=D + 1)
            nc.vector.memset(v4[:, :, :, D : D + 1], 1.0)

            def ln_project(act_sb, w_sb, raw, mv, ntiles, scale_fold, bias_nm):
                """act_sb: (P, CT, S) transposed activations; produces raw
                (P, ntiles, G) = act.T @ W + bias and per-head mean/var."""
                for it in range(ntiles):
                    ps = psA.tile([P, G], F32, tag="psA")
                    for ct in range(CT):
                        nc.tensor.matmul(
                            ps,
                            lhsT=act_sb[:, ct, it * P : (it + 1) * P],
                            rhs=w_sb[:, ct, :],
                            start=(ct == 0),
                            stop=(ct == CT - 1),
                        )
                    dst = raw[:, it, :]
                    if bias_nm in vec_sb:
                        nc.vector.tensor_add(out=dst, in0=ps, in1=vec_sb[bias_nm])
                    else:
                        nc.vector.tensor_copy(out=dst, in_=ps)
                    for h4 in range(4):
                        st = work.tile([P, 6], F32, tag="bnst")
                        nc.vector.bn_stats(out=st, in_=dst[:, h4 * D : (h4 + 1) * D])
                        nc.vector.bn_aggr(out=mv[:, it, h4, :], in_=st)
                # batched rstd: rs = scale_fold / sqrt(var + eps)
                n4 = ntiles * 4
                mv_flat = mv.rearrange("p i h s -> p (i h s)")
                sd = work.tile([P, n4], F32, tag=f"sd{bias_nm}")
                nc.scalar.activation(
                    out=sd, in_=mv_flat[:, 1::2], func=AF.Sqrt, bias=eps_sb, scale=1.0
                )
                rs = work.tile([P, n4], F32, tag=f"rs{bias_nm}")
                nc.vector.reciprocal(out=rs, in_=sd)
                if scale_fold != 1.0:
                    nc.vector.tensor_scalar_mul(out=rs, in0=rs, scalar1=scale_fold)
                nm_ = work.tile([P, n4], F32, tag=f"nm{bias_nm}")
                nc.vector.tensor_mul(out=nm_, in0=mv_flat[:, 0::2], in1=rs)
                nc.vector.tensor_scalar_mul(out=nm_, in0=nm_, scalar1=-1.0)
                return rs, nm_

            def ln_apply_transpose(raw, rs, nm_, ntiles, w_nm, b_nm, dstT):
                for it in range(ntiles):
                    qa = work.tile([P, G], TR_DT, tag="qa")
                    for h4 in range(4):
                        i4 = it * 4 + h4
                        nc.vector.tensor_scalar(
                            out=qa[:, h4 * D : (h4 + 1) * D],
                            in0=raw[:, it, h4 * D : (h4 + 1) * D],
                            scalar1=rs[:, i4 : i4 + 1],
                            scalar2=nm_[:, i4 : i4 + 1],
                            op0=mybir.AluOpType.mult,
                            op1=mybir.AluOpType.add,
                        )
                    if w_nm in vec_sb:
                        nc.vector.tensor_mul(out=qa, in0=qa, in1=vec_sb[w_nm])
                    if b_nm in vec_sb:
                        nc.vector.tensor_add(out=qa, in0=qa, in1=vec_sb[b_nm])
                    for half in range(2):
                        pt = psT.tile([P, P], TR_DT, tag="ptr")
                        nc.tensor.transpose(pt, qa[:, half * P : (half + 1) * P], ident)
                        nc.vector.tensor_copy(
                            out=dstT[half][:, it * P : (it + 1) * P], in_=pt
                        )

            # ------- Phase 1+2: q/k/v projections + LN + transposes ----------
            # One scope so the Tile scheduler interleaves q-, k- and v-side
            # matmuls with the DVE LayerNorm work: PE stays dense (HAM warm).
            with (
                tc.tile_pool(name="ph1", bufs=1) as ph1,
                tc.tile_pool(name="work", bufs=3) as work,
                tc.tile_pool(name="psA", bufs=3, space="PSUM") as psA,
                tc.tile_pool(name="psT", bufs=2, space="PSUM") as psT,
            ):
                xT_sb = ph1.tile([P, CT, S1], MM_DT, tag="xTs")
                xv = xT_d[:, :].rearrange("(ct p) i -> ct p i", p=P)
                for ct in range(CT):
                    nc.sync.dma_start(out=xT_sb[:, ct, :], in_=xv[ct])
                wq_sb = ph1.tile([P, CT, G], MM_DT, tag="wqs")
                wqv = wqT_d[:, :].rearrange("(ct p) g -> ct p g", p=P)
                for ct in range(CT):
                    nc.sync.dma_start(out=wq_sb[:, ct, :], in_=wqv[ct])
                yT_sb = ph1.tile([P, CT, S2P], MM_DT, tag="yTs")
                yv = yT_d[:, :].rearrange("(ct p) j -> ct p j", p=P)
                for ct in range(CT):
                    nc.sync.dma_start(out=yT_sb[:, ct, :], in_=yv[ct])
                wk_sb = ph1.tile([P, CT, G], MM_DT, tag="wks")
                wv_sb = ph1.tile([P, CT, G], MM_DT, tag="wvs")
                wkv = wkT_d[:, :].rearrange("(ct p) g -> ct p g", p=P)
                wvv = wvT_d[:, :].rearrange("(ct p) g -> ct p g", p=P)
                for ct in range(CT):
                    nc.sync.dma_start(out=wk_sb[:, ct, :], in_=wkv[ct])
                    nc.sync.dma_start(out=wv_sb[:, ct, :], in_=wvv[ct])

                qraw = ph1.tile([P, NI, G], F32, tag="qraw")
                mvq = ph1.tile([P, NI, 4, 2], F32, tag="mvq")
                rs_q, nm_q = ln_project(
                    xT_sb, wq_sb, qraw, mvq, NI, 1.0 / math.sqrt(D), "bq"
                )
                kraw = ph1.tile([P, NJ, G], F32, tag="kraw")
                mvk = ph1.tile([P, NJ, 4, 2], F32, tag="mvk")
                rs_k, nm_k = ln_project(yT_sb, wk_sb, kraw, mvk, NJ, 1.0, "bk")

                # v projection (no LN, no transpose; strided 65-col layout)
                for jt in range(NJ):
                    ps = psA.tile([P, G], F32, tag="psA")
                    for ct in range(CT):
                        nc.tensor.matmul(
                            ps,
                            lhsT=yT_sb[:, ct, jt * P : (jt + 1) * P],
                            rhs=wv_sb[:, ct, :],
                            start=(ct == 0),
                            stop=(ct == CT - 1),
                        )
                    ps3 = ps.rearrange("p (h e) -> p h e", e=D)
                    vdst = v4[:, jt, :, 0:D]
                    if "bv" in vec_sb:
                        bv3 = vec_sb["bv"].rearrange("p (h e) -> p h e", e=D)
                        nc.vector.tensor_add(out=vdst, in0=ps3, in1=bv3)
                    else:
                        nc.vector.tensor_copy(out=vdst, in_=ps3)

                ln_apply_transpose(qraw, rs_q, nm_q, NI, "qw", "qb", qT)
                ln_apply_transpose(kraw, rs_k, nm_k, NJ, "kw", "kb", kT)

            # ---------------- Phase 3: attention ----------------------------
            with (
                tc.tile_pool(name="pp", bufs=2) as ppool,
                tc.tile_pool(name="attw", bufs=3) as attw,
                tc.tile_pool(name="attden", bufs=1) as attden,
                tc.tile_pool(name="dram", bufs=1, space="DRAM") as dramp,
                tc.tile_pool(name="psS", bufs=2, space="PSUM") as psS,
                tc.tile_pool(name="psC", bufs=1, space="PSUM") as psC,
            ):
                NU = NIB * 2 * 2 * NC2  # normalization units (h2 x cc per ib,hp)
                rec_dram = dramp.tile([NU, 512], F32, tag="rec_dram")
                for ib in range(NIB):
                    for hp in range(2):
                        pts = [
                            ppool.tile([P, NJ, IBW], PV_DT, tag=f"p{h2}", name=f"p{h2}")
                            for h2 in range(2)
                        ]
                        for jt in range(NJ):
                            for h2 in range(2):
                                ps = psS.tile([P, IBW], F32, tag="ps_s")
                                for cc in range(NC2):
                                    c0 = ib * IBW + cc * 512
                                    nc.tensor.matmul(
                                        ps[:, cc * 512 : (cc + 1) * 512],
                                        lhsT=kT[hp][
                                            h2 * D : (h2 + 1) * D,
                                            jt * P : (jt + 1) * P,
                                        ],
                                        rhs=qT[hp][
                                            h2 * D : (h2 + 1) * D, c0 : c0 + 512
                                        ],
                                        start=True,
                                        stop=True,
                                    )
                                nc.scalar.activation(
                                    out=pts[h2][:, jt, :],
                                    in_=ps,
                                    func=AF.Exp,
                                    bias=mask_sb[:, jt : jt + 1],
                                    scale=1.0,
                                )
                        pcs = {}
                        cus = {}
                        u0 = (ib * 2 + hp) * 2 * NC2
                        den_blk = attw.tile([P, 512], F32, tag="den_blk")
                        nc.vector.memset(den_blk, 1.0)
                        for h2 in range(2):
                            hg = hp * 2 + h2
                            for cc in range(NC2):
                                pc = psC.tile(
                                    [D + 1, 512], F32, tag=f"ps_c{h2}{cc}", name="pc"
                                )
                                pcs[(h2, cc)] = pc
                                for jt in range(NJ):
                                    nc.tensor.matmul(
                                        pc,
                                        lhsT=v_sb[
                                            :, jt, hg * (D + 1) : (hg + 1) * (D + 1)
                                        ],
                                        rhs=pts[h2][:, jt, cc * 512 : (cc + 1) * 512],
                                        start=(jt == 0),
                                        stop=(jt == NJ - 1),
                                    )
                                uu = h2 * NC2 + cc
                                nc.vector.tensor_copy(
                                    out=den_blk[32 * uu : 32 * uu + 1, :],
                                    in_=pc[D : D + 1, :],
                                )
                                cu = attw.tile(
                                    [D, 512], F32, tag=f"cu{uu}", name="cu"
                                )
                                nc.vector.tensor_copy(out=cu, in_=pc[0:D, :])
                                cus[(h2, cc)] = cu
                        # one batched reciprocal per (ib, hp) block (rows at
                        # 32-aligned partitions), bounced via DRAM so each row
                        # can partition-broadcast on the way back in
                        rec_blk = attw.tile([P, 512], F32, tag="rec_blk")
                        nc.vector.reciprocal(out=rec_blk, in_=den_blk)
                        rec_rows = rec_blk.rearrange("(a b) f -> a b f", b=32)[:, 0, :]
                        nc.gpsimd.dma_start(
                            out=rec_dram[u0 : u0 + 2 * NC2, :], in_=rec_rows
                        )
                        for h2 in range(2):
                            for cc in range(NC2):
                                u = u0 + h2 * NC2 + cc
                                rec = attw.tile([D, 512], F32, tag="rec")
                                nc.gpsimd.dma_start(
                                    out=rec, in_=_bcast_row(rec_dram[u : u + 1, :], D)
                                )
                                c0 = ib * IBW + cc * 512
                                nc.vector.tensor_mul(
                                    out=ctxT[hp][h2 * D : (h2 + 1) * D, c0 : c0 + 512],
                                    in0=cus[(h2, cc)],
                                    in1=rec,
                                )

            # ---------------- Phase 4: output projection --------------------
            with (
                tc.tile_pool(name="ph4", bufs=1) as ph4,
                tc.tile_pool(name="ow", bufs=3) as ow,
                tc.tile_pool(name="psO", bufs=2, space="PSUM") as psO,
            ):
                wo_sb = ph4.tile([P, 2, C], MM_DT, tag="wo")
                wov = woT_d[:, :].rearrange("(k p) c -> k p c", p=P)
                for kt in range(2):
                    nc.sync.dma_start(out=wo_sb[:, kt, :], in_=wov[kt])
                for it in range(NI):
                    po = psO.tile([P, C], F32, tag="ps_o")
                    for oc in range(C // 512):
                        for kt in range(2):
                            nc.tensor.matmul(
                                po[:, oc * 512 : (oc + 1) * 512],
                                lhsT=ctxT[kt][:, it * P : (it + 1) * P],
                                rhs=wo_sb[:, kt, oc * 512 : (oc + 1) * 512].bitcast(
                                    MM_DT
                                ),
                                start=(kt == 0),
                                stop=(kt == 1),
                            )
                    ot = ow.tile([P, C], F32, tag="ot")
                    nc.vector.tensor_copy(out=ot, in_=po)
                    nc.sync.dma_start(out=out_d[it * P : (it + 1) * P, :], in_=ot)

    nc.finalize()
    return nc


def kernel(x, y, padding_mask, Wq, bq, Wkv, bkv, qn_w, qn_b, kn_w, kn_b, Wo, bo):
    global LAST_EXEC_NS
    x = np.asarray(x, dtype=np.float32)
    y = np.asarray(y, dtype=np.float32)
    padding_mask = np.asarray(padding_mask)
    Wq = np.asarray(Wq, dtype=np.float32)
    bq = np.asarray(bq, dtype=np.float32)
    Wkv = np.asarray(Wkv, dtype=np.float32)
    bkv = np.asarray(bkv, dtype=np.float32)
    qn_w = np.asarray(qn_w, dtype=np.float32)
    qn_b = np.asarray(qn_b, dtype=np.float32)
    kn_w = np.asarray(kn_w, dtype=np.float32)
    kn_b = np.asarray(kn_b, dtype=np.float32)
    Wo = np.asarray(Wo, dtype=np.float32)
    bo = np.asarray(bo, dtype=np.float32)

    b, S1, C = x.shape
    assert b == 2 and C % 16 == 0
    d = C // 16
    scale = d ** -0.5
    G = 4 * d  # 4 heads per core

    idxs = [np.flatnonzero(padding_mask[bi]) for bi in range(b)]
    s2v = [len(ix) for ix in idxs]
    S2P = max(P, ((max(s2v) + P - 1) // P) * P)

    flags = {
        "bq": bool(np.any(bq)),
        "bk": bool(np.any(bkv[:C])),
        "bv": bool(np.any(bkv[C:])),
        "qw": not bool(np.all(qn_w == 1.0)),
        "qb": bool(np.any(qn_b)),
        "kw": not bool(np.all(kn_w == 1.0)),
        "kb": bool(np.any(kn_b)),
    }

    nc = _build_nc(S1, S2P, C, flags)

    mm_np = {BF16: ml_dtypes.bfloat16, F32R: np.float32, F32: np.float32}[MM_DT]
    in_maps = []
    yTs = []
    for bi in range(b):
        yv = np.zeros((S2P, C), np.float32)
        yv[: s2v[bi]] = y[bi][idxs[bi]]
        yTs.append(np.ascontiguousarray(yv.T).astype(mm_np))
    xTs = [np.ascontiguousarray(x[bi].T).astype(mm_np) for bi in range(b)]
    for core in range(8):
        bc, g = divmod(core, 4)
        rows = slice(g * G, (g + 1) * G)
        vecs = np.zeros((8, G), np.float32)
        vecs[0] = bq[rows]
        vecs[1] = bkv[rows]
        vecs[2] = bkv[C + g * G : C + (g + 1) * G]
        vecs[3] = np.tile(qn_w, 4)
        vecs[4] = np.tile(qn_b * scale, 4)
        vecs[5] = np.tile(kn_w, 4)
        vecs[6] = np.tile(kn_b, 4)
        mb = np.zeros((S2P,), np.float32)
        mb[s2v[bc] :] = MASK_NEG
        in_maps.append(
            {
                "xT": xTs[bc],
                "yT": yTs[bc],
                "wqT": np.ascontiguousarray(Wq[rows, :].T).astype(mm_np),
                "wkT": np.ascontiguousarray(Wkv[rows, :].T).astype(mm_np),
                "wvT": np.ascontiguousarray(
                    Wkv[C + g * G : C + (g + 1) * G, :].T
                ).astype(mm_np),
                "woT": np.ascontiguousarray(Wo[:, rows].T).astype(mm_np),
                "vec": vecs,
                "maskb": mb,
            }
        )

    res = run_bass_kernel_spmd(nc, in_maps, core_ids=list(range(8)))
    LAST_EXEC_NS = res.exec_time_ns

    out = np.zeros((b, S1, C), np.float32)
    for core in range(8):
        out[core // 4] += res.results[core]["out"]
    out += bo
    return out
